# revision 11
# baseline (speedup 1.0000x reference)
"""Causal self-attention (B=4, T=2048, C=1024, H=16) on 8 TRN2 NeuronCores.

Sharding: core = (batch b, head-group hg). Data parallel over B (4), tensor
parallel over heads (2 groups of 8). Each core computes a partial output
projection for its 8 heads; the host sums the two partials per batch
(row-parallel linear unshard).

Per-core pipeline (all matmuls fp32r, accumulate fp32 in PSUM):
  0) PE-transpose x [T,C] -> xT [C,T] (contraction for qkv is over C)
  1) kT = wk^T xT ([512, T], head h at partition rows (h%2)*64..),
     qT same but stored zero-PADDED per head: qT_pad[:, h, :] has head h's
     64 dims in its partition half and zeros in the other half, so the
     scores matmul can run with K=128 (full PE rows; the pad half multiplies
     the other head's kT rows by zero). K=64 matmuls starve the PE HAM
     activity monitor and the clock gates down to 1.2 GHz.
     v = x wv ([T, 512]) stored per (head, t-tile) with a ones column
     appended -> v_aug [128k, 65]
  2) per head, per 512-wide q block: scoresT [128k, 512q] = kT_blk^T @ qT_blk
     (transposed layout so the softmax denominator comes from the PE via the
     ones column of v_aug instead of a cross-partition reduce),
     p = exp(scoresT/32) (no max subtraction: |scores| <= ~2.1), exp batched
     3 blocks per ACTIVATE over a [128,1536] 3-bank psum window to amortize
     the 352-cycle ACT fixed overhead,
     causal: skip blocks above the diagonal, trim + triangular-mask the 4
     diagonal blocks,
     yT_aug [65, 512q] += v_aug^T @ p  (row 64 = softmax denominators),
     yT = yT_aug[0:64] * (1/denominator broadcast across partitions)
  3) out_partial [T, 1024] = yT_all^T @ wp, accumulated over 4 k-tiles
"""
import numpy as np
from contextlib import ExitStack

import concourse.bass as bass
import concourse.mybir as mybir
import concourse.tile as tile
from concourse import bacc
from concourse.bass_utils import run_bass_kernel_spmd
from concourse.masks import make_identity

F32 = mybir.dt.float32
F32R = mybir.dt.float32r
AF = mybir.ActivationFunctionType
F16 = mybir.dt.float16

T = 2048
C = 1024
H_PER_CORE = 8          # heads per core
D = 64                  # head dim
GC = H_PER_CORE * D     # 512 channels per head-group
SCALE = 1.0 / 32.0      # C ** -0.5
N_CORES = 8


def build(nc):
    x_d = nc.dram_tensor("x", [T, C], F32R, kind="ExternalInput").ap()
    wq_d = nc.dram_tensor("wq", [C, GC], F32R, kind="ExternalInput").ap()
    wk_d = nc.dram_tensor("wk", [C, GC], F32R, kind="ExternalInput").ap()
    wv_d = nc.dram_tensor("wv", [C, GC], F32R, kind="ExternalInput").ap()
    wp_d = nc.dram_tensor("wp", [GC, C], F32R, kind="ExternalInput").ap()
    out_d = nc.dram_tensor("out", [T, C], F32, kind="ExternalOutput").ap()

    NT = T // 128        # 16 t-tiles
    NC_ = C // 128       # 8 c-tiles
    NQSB = T // 512      # 4 q superblocks

    with tile.TileContext(nc) as tc, ExitStack() as ctx:
        const = ctx.enter_context(tc.tile_pool(name="const", bufs=1))
        persist = ctx.enter_context(tc.tile_pool(name="persist", bufs=1))

        ident_f32 = const.tile([128, 128], F32)
        make_identity(nc, ident_f32[:])
        ident = const.tile([128, 128], F32R)
        nc.vector.tensor_copy(ident[:], ident_f32[:])
        # tri_mask[k, j] = 1.0 if k <= j else 0.0
        tri_mask = const.tile([128, 128], F16)
        nc.gpsimd.memset(tri_mask[:], 1.0)
        nc.gpsimd.affine_select(
            out=tri_mask[:], in_=tri_mask[:],
            compare_op=mybir.AluOpType.is_ge, fill=0.0, base=0,
            pattern=[[1, 128]], channel_multiplier=-1,
        )

        # persistent activations
        qT_pad = persist.tile([128, H_PER_CORE, T], F32R)  # [head, t], zero-padded
        kT_sb = persist.tile([128, 4, T], F32R)            # [m-tile, t]
        v_aug = persist.tile([128, H_PER_CORE, NT, 65], F16)
        yT_sb = persist.tile([128, 4, T], F32R)
        nc.gpsimd.memset(qT_pad[:].bitcast(F32), 0.0)
        nc.gpsimd.memset(v_aug[:, :, :, 64], 1.0)

        copy_engines = [nc.vector.tensor_copy, nc.scalar.copy]
        cp_idx = 0

        def copy_any(dst, src):
            nonlocal cp_idx
            copy_engines[cp_idx % 2](dst, src)
            cp_idx += 1

        # ---- phases 0 + 1, in two T halves to bound xT footprint ----
        with ExitStack() as p01:
            xa_pool = p01.enter_context(tc.tile_pool(name="xa", bufs=2))
            xT_pool = p01.enter_context(tc.tile_pool(name="xT", bufs=1))
            w_pool = p01.enter_context(tc.tile_pool(name="w", bufs=8))
            psT = p01.enter_context(tc.tile_pool(name="psT", bufs=2, space="PSUM"))
            psQK = p01.enter_context(tc.tile_pool(name="psQK", bufs=4, space="PSUM"))

            for th in range(2):
                TH = T // 2  # 1024 t per half
                xT = xT_pool.tile([128, NC_, TH], F32R, tag="xT")
                # phase 0: transpose this half of x
                for tt8 in range(8):
                    tt = th * 8 + tt8
                    xa = xa_pool.tile([128, C], F32R)
                    nc.sync.dma_start(xa[:], x_d[tt * 128:(tt + 1) * 128, :])
                    for cq in range(2):      # 4 transposes per psum bank
                        pt = psT.tile([128, 512], F32R)
                        for j in range(4):
                            cb = cq * 4 + j
                            nc.tensor.transpose(
                                pt[:, j * 128:(j + 1) * 128],
                                xa[:, cb * 128:(cb + 1) * 128], ident[:])
                        # strided copy into xT: [128, 4 c-planes, 128 t]
                        copy_any(
                            xT[:, cq * 4:(cq + 1) * 4, tt8 * 128:(tt8 + 1) * 128],
                            pt[:].rearrange("p (c t) -> p c t", c=4))

                # phase 1a: qT (padded layout) / kT for this half
                for wd, is_q in ((wq_d, True), (wk_d, False)):
                    w_tiles = []
                    for ct in range(NC_):
                        wt = w_pool.tile([128, GC], F32R, tag="w")
                        nc.sync.dma_start(wt[:], wd[ct * 128:(ct + 1) * 128, :])
                        w_tiles.append(wt)
                    for mt in range(4):
                        for nt in range(2):   # 512-wide t chunks in this half
                            pq = psQK.tile([128, 512], F32, tag="qkv")
                            for ct in range(NC_):
                                nc.tensor.matmul(
                                    pq[:],
                                    w_tiles[ct][:, mt * 128:(mt + 1) * 128],
                                    xT[:, ct, nt * 512:(nt + 1) * 512],
                                    start=(ct == 0), stop=(ct == NC_ - 1))
                            span = slice(th * TH + nt * 512, th * TH + (nt + 1) * 512)
                            if is_q:
                                # head 2mt -> partitions 0:64 of slot 2mt,
                                # head 2mt+1 -> partitions 64:128 of slot 2mt+1
                                copy_any(qT_pad[0:64, 2 * mt, span], pq[0:64, :])
                                copy_any(qT_pad[64:128, 2 * mt + 1, span], pq[64:128, :])
                            else:
                                copy_any(kT_sb[:, mt, span], pq[:])

                # phase 1b: v for this half (natural layout, into v_aug)
                wv_tiles = []
                for ct in range(NC_):
                    wt = w_pool.tile([128, GC], F32R, tag="w")
                    nc.sync.dma_start(wt[:], wv_d[ct * 128:(ct + 1) * 128, :])
                    wv_tiles.append(wt)
                for tt8 in range(8):
                    tt = th * 8 + tt8
                    pv = psQK.tile([128, 512], F32, tag="qkv")
                    for ct in range(NC_):
                        nc.tensor.matmul(
                            pv[:],
                            xT[:, ct, tt8 * 128:(tt8 + 1) * 128],
                            wv_tiles[ct][:],
                            start=(ct == 0), stop=(ct == NC_ - 1))
                    copy_any(
                        v_aug[:, :, tt, 0:64],
                        pv[:].rearrange("p (h d) -> p h d", h=H_PER_CORE))

        # preload the proj weights before attention so phase 3 starts hot
        wp_pool = ctx.enter_context(tc.tile_pool(name="wp", bufs=1))
        wp_sb = wp_pool.tile([128, 4, C], F32R)
        for kt in range(4):
            nc.sync.dma_start(wp_sb[:, kt, :], wp_d[kt * 128:(kt + 1) * 128, :])

        # ---- phase 2: attention ----
        with ExitStack() as p2:
            pT_pool = p2.enter_context(tc.tile_pool(name="pT", bufs=3))
            rc_pool = p2.enter_context(tc.tile_pool(name="rc", bufs=3))
            rb_pool = p2.enter_context(tc.tile_pool(name="rb", bufs=2))
            psS = p2.enter_context(tc.tile_pool(name="psS", bufs=2, space="PSUM"))
            psY = p2.enter_context(tc.tile_pool(name="psY", bufs=2, space="PSUM"))

            for h in range(H_PER_CORE):
                mt_h = h // 2
                for qsb in range(NQSB):
                    nkb = 4 * (qsb + 1)
                    yTp = psY.tile([65, 512], F32, tag="yTp")
                    # Diagonal blocks FIRST (their exps overlap the prefix
                    # matmuls instead of stalling the q-block tail), paired
                    # two per wide psum slot; then full-width prefix blocks
                    # in 3s with one batched exp each.
                    diag = [(kb, kb * 128 - qsb * 512)
                            for kb in range(4 * qsb, nkb)]
                    prefix = [(kb, 0) for kb in range(4 * qsb)]
                    groups = [diag[0:2], diag[2:4]]
                    groups += [prefix[i:i + 3] for i in range(0, len(prefix), 3)]
                    n_pv = 0
                    for g in groups:
                        wide = psS.tile([128, 1536], F32, tag="wide")
                        pTw = pT_pool.tile([128, 1536], F16, tag="pTw")
                        diag_group = g[0][0] >= 4 * qsb
                        for i, (kb, lo) in enumerate(g):
                            nc.tensor.matmul(
                                wide[:, i * 512 + lo:(i + 1) * 512],
                                kT_sb[:, mt_h, kb * 128:(kb + 1) * 128],
                                qT_pad[:, h, qsb * 512 + lo:(qsb + 1) * 512],
                                start=True, stop=True)
                        if diag_group:
                            # per-block exp, width-trimmed to the causal range
                            for i, (kb, lo) in enumerate(g):
                                nc.scalar.activation(
                                    pTw[:, i * 512 + lo:(i + 1) * 512],
                                    wide[:, i * 512 + lo:(i + 1) * 512],
                                    AF.Exp, bias=0.0, scale=SCALE)
                        else:
                            nc.scalar.activation(
                                pTw[:, 0:len(g) * 512], wide[:, 0:len(g) * 512],
                                AF.Exp, bias=0.0, scale=SCALE)
                        for i, (kb, lo) in enumerate(g):
                            if diag_group:
                                # zero the strictly-upper triangle
                                nc.vector.tensor_mul(
                                    pTw[:, i * 512 + lo:i * 512 + lo + 128],
                                    pTw[:, i * 512 + lo:i * 512 + lo + 128],
                                    tri_mask[:])
                            nc.tensor.matmul(
                                yTp[:, lo:512],
                                v_aug[:, h, kb, :],
                                pTw[:, i * 512 + lo:(i + 1) * 512],
                                start=(n_pv == 0), stop=(n_pv == nkb - 1))
                            n_pv += 1
                    recip = rc_pool.tile([1, 512], F32)
                    nc.vector.reciprocal(recip[:], yTp[64:65, :])
                    rbc = rb_pool.tile([64, 512], F32)
                    nc.gpsimd.partition_broadcast(rbc[:], recip[:])
                    nc.vector.tensor_mul(
                        yT_sb[64 * (h % 2):64 * (h % 2) + 64, mt_h,
                              qsb * 512:(qsb + 1) * 512],
                        yTp[0:64, :], rbc[:])

        # ---- phase 3: output projection (partial; host sums head groups) ----
        with ExitStack() as p3:
            so_pool = p3.enter_context(tc.tile_pool(name="so", bufs=3))
            psO = p3.enter_context(tc.tile_pool(name="psO", bufs=4, space="PSUM"))
            for mt in range(NT):
                for n2 in range(2):
                    pp = psO.tile([128, 512], F32)
                    for kt in range(4):
                        nc.tensor.matmul(
                            pp[:],
                            yT_sb[:, kt, mt * 128:(mt + 1) * 128],
                            wp_sb[:, kt, n2 * 512:(n2 + 1) * 512],
                            start=(kt == 0), stop=(kt == 3))
                    so = so_pool.tile([128, 512], F32)
                    copy_any(so[:], pp[:])
                    nc.sync.dma_start(
                        out_d[mt * 128:(mt + 1) * 128, n2 * 512:(n2 + 1) * 512],
                        so[:])


_CACHE = {}


def _get_nc():
    if "nc" not in _CACHE:
        nc = bacc.Bacc("TRN2", target_bir_lowering=False, debug=False,
                       num_devices=N_CORES)
        build(nc)
        nc.compile()
        _CACHE["nc"] = nc
    return _CACHE["nc"]


def make_in_maps(x, w_attn, w_proj):
    x = np.asarray(x, dtype=np.float32)
    w_attn = np.asarray(w_attn, dtype=np.float32)
    w_proj = np.asarray(w_proj, dtype=np.float32)
    in_maps = []
    for core in range(N_CORES):
        b, hg = divmod(core, 2)
        cs = slice(hg * GC, (hg + 1) * GC)
        in_maps.append({
            "x": np.ascontiguousarray(x[b]),
            "wq": np.ascontiguousarray(w_attn[:, 0 * C:1 * C][:, cs]),
            "wk": np.ascontiguousarray(w_attn[:, 1 * C:2 * C][:, cs]),
            "wv": np.ascontiguousarray(w_attn[:, 2 * C:3 * C][:, cs]),
            "wp": np.ascontiguousarray(w_proj[cs, :]),
        })
    return in_maps


def kernel(x, w_attn, w_proj, _trace=False, _trace_kwargs=None):
    nc = _get_nc()
    in_maps = make_in_maps(x, w_attn, w_proj)
    res = run_bass_kernel_spmd(nc, in_maps, core_ids=list(range(N_CORES)),
                               trace=_trace, **(_trace_kwargs or {}))
    _CACHE["last_results"] = res
    B = np.asarray(x).shape[0]
    out = np.empty((B, T, C), dtype=np.float32)
    for b in range(B):
        out[b] = res.results[2 * b]["out"] + res.results[2 * b + 1]["out"]
    return out


# revision 12
# speedup vs baseline: 1.1994x; 1.1994x over previous
"""Causal self-attention (B=4, T=2048, C=1024, H=16) on 8 TRN2 NeuronCores.

Sharding: core = (batch b, head-group hg). Data parallel over B (4), tensor
parallel over heads (2 groups of 8). Each core computes a partial output
projection for its 8 heads; the host sums the two partials per batch
(row-parallel linear unshard).

Per-core pipeline (all matmuls fp32r, accumulate fp32 in PSUM):
  0) PE-transpose x [T,C] -> xT [C,T] (contraction for qkv is over C)
  1) kT = wk^T xT ([512, T], head h at partition rows (h%2)*64..),
     qT same but stored zero-PADDED per head: qT_pad[:, h, :] has head h's
     64 dims in its partition half and zeros in the other half, so the
     scores matmul can run with K=128 (full PE rows; the pad half multiplies
     the other head's kT rows by zero). K=64 matmuls starve the PE HAM
     activity monitor and the clock gates down to 1.2 GHz.
     v = x wv ([T, 512]) stored per (head, t-tile) with a ones column
     appended -> v_aug [128k, 65]
  2) per head, per 512-wide q block: scoresT [128k, 512q] = kT_blk^T @ qT_blk
     (transposed layout so the softmax denominator comes from the PE via the
     ones column of v_aug instead of a cross-partition reduce),
     p = exp(scoresT/32) (no max subtraction: |scores| <= ~2.1), exp batched
     3 blocks per ACTIVATE over a [128,1536] 3-bank psum window to amortize
     the 352-cycle ACT fixed overhead,
     causal: skip blocks above the diagonal, trim + triangular-mask the 4
     diagonal blocks,
     yT_aug [65, 512q] += v_aug^T @ p  (row 64 = softmax denominators),
     yT = yT_aug[0:64] * (1/denominator broadcast across partitions)
  3) out_partial [T, 1024] = yT_all^T @ wp, accumulated over 4 k-tiles
"""
import numpy as np
from contextlib import ExitStack

import concourse.bass as bass
import concourse.mybir as mybir
import concourse.tile as tile
from concourse import bacc
from concourse.bass_utils import run_bass_kernel_spmd
from concourse.masks import make_identity

F32 = mybir.dt.float32
F32R = mybir.dt.float32r
AF = mybir.ActivationFunctionType
F16 = mybir.dt.float16

T = 2048
C = 1024
H_PER_CORE = 8          # heads per core
D = 64                  # head dim
GC = H_PER_CORE * D     # 512 channels per head-group
SCALE = 1.0 / 32.0      # C ** -0.5
N_CORES = 8


def build(nc):
    x_d = nc.dram_tensor("x", [T, C], F32R, kind="ExternalInput").ap()
    wq_d = nc.dram_tensor("wq", [C, GC], F32R, kind="ExternalInput").ap()
    wk_d = nc.dram_tensor("wk", [C, GC], F32R, kind="ExternalInput").ap()
    wv_d = nc.dram_tensor("wv", [C, GC], F32R, kind="ExternalInput").ap()
    wp_d = nc.dram_tensor("wp", [GC, C], F32R, kind="ExternalInput").ap()
    out_d = nc.dram_tensor("out", [T, C], F32, kind="ExternalOutput").ap()

    NT = T // 128        # 16 t-tiles
    NC_ = C // 128       # 8 c-tiles
    NQSB = T // 512      # 4 q superblocks

    with tile.TileContext(nc) as tc, ExitStack() as ctx:
        const = ctx.enter_context(tc.tile_pool(name="const", bufs=1))
        persist = ctx.enter_context(tc.tile_pool(name="persist", bufs=1))

        ident_f32 = const.tile([128, 128], F32)
        make_identity(nc, ident_f32[:])
        ident = const.tile([128, 128], F32R)
        nc.vector.tensor_copy(ident[:], ident_f32[:])
        # tri_mask[k, j] = 1.0 if k <= j else 0.0
        tri_mask = const.tile([128, 128], F16)
        nc.gpsimd.memset(tri_mask[:], 1.0)
        nc.gpsimd.affine_select(
            out=tri_mask[:], in_=tri_mask[:],
            compare_op=mybir.AluOpType.is_ge, fill=0.0, base=0,
            pattern=[[1, 128]], channel_multiplier=-1,
        )

        # persistent activations
        qT_pad = persist.tile([128, H_PER_CORE, T], F32R)  # [head, t], zero-padded
        kT_sb = persist.tile([128, 4, T], F32R)            # [m-tile, t]
        v_aug = persist.tile([128, H_PER_CORE, NT, 65], F16)
        yT_sb = persist.tile([128, 4, T], F32R)
        nc.gpsimd.memset(qT_pad[:].bitcast(F32), 0.0)
        nc.gpsimd.memset(v_aug[:, :, :, 64], 1.0)

        copy_engines = [nc.vector.tensor_copy, nc.scalar.copy]
        cp_idx = 0

        def copy_any(dst, src):
            nonlocal cp_idx
            copy_engines[cp_idx % 2](dst, src)
            cp_idx += 1

        # ---- phases 0 + 1, in two T halves to bound xT footprint ----
        with ExitStack() as p01:
            xa_pool = p01.enter_context(tc.tile_pool(name="xa", bufs=2))
            xT_pool = p01.enter_context(tc.tile_pool(name="xT", bufs=1))
            w_pool = p01.enter_context(tc.tile_pool(name="w", bufs=8))
            psT = p01.enter_context(tc.tile_pool(name="psT", bufs=2, space="PSUM"))
            psQK = p01.enter_context(tc.tile_pool(name="psQK", bufs=4, space="PSUM"))

            for th in range(2):
                TH = T // 2  # 1024 t per half
                xT = xT_pool.tile([128, NC_, TH], F32R, tag="xT")
                # phase 0: transpose this half of x
                for tt8 in range(8):
                    tt = th * 8 + tt8
                    xa = xa_pool.tile([128, C], F32R)
                    nc.sync.dma_start(xa[:], x_d[tt * 128:(tt + 1) * 128, :])
                    for cq in range(2):      # 4 transposes per psum bank
                        pt = psT.tile([128, 512], F32R)
                        for j in range(4):
                            cb = cq * 4 + j
                            nc.tensor.transpose(
                                pt[:, j * 128:(j + 1) * 128],
                                xa[:, cb * 128:(cb + 1) * 128], ident[:])
                        # strided copy into xT: [128, 4 c-planes, 128 t]
                        copy_any(
                            xT[:, cq * 4:(cq + 1) * 4, tt8 * 128:(tt8 + 1) * 128],
                            pt[:].rearrange("p (c t) -> p c t", c=4))

                # phase 1a: qT (padded layout) / kT for this half
                for wd, is_q in ((wq_d, True), (wk_d, False)):
                    w_tiles = []
                    for ct in range(NC_):
                        wt = w_pool.tile([128, GC], F32R, tag="w")
                        nc.sync.dma_start(wt[:], wd[ct * 128:(ct + 1) * 128, :])
                        w_tiles.append(wt)
                    for mt in range(4):
                        for nt in range(2):   # 512-wide t chunks in this half
                            pq = psQK.tile([128, 512], F32, tag="qkv")
                            for ct in range(NC_):
                                nc.tensor.matmul(
                                    pq[:],
                                    w_tiles[ct][:, mt * 128:(mt + 1) * 128],
                                    xT[:, ct, nt * 512:(nt + 1) * 512],
                                    start=(ct == 0), stop=(ct == NC_ - 1))
                            span = slice(th * TH + nt * 512, th * TH + (nt + 1) * 512)
                            if is_q:
                                # head 2mt -> partitions 0:64 of slot 2mt,
                                # head 2mt+1 -> partitions 64:128 of slot 2mt+1
                                copy_any(qT_pad[0:64, 2 * mt, span], pq[0:64, :])
                                copy_any(qT_pad[64:128, 2 * mt + 1, span], pq[64:128, :])
                            else:
                                copy_any(kT_sb[:, mt, span], pq[:])

                # phase 1b: v for this half (natural layout, into v_aug)
                wv_tiles = []
                for ct in range(NC_):
                    wt = w_pool.tile([128, GC], F32R, tag="w")
                    nc.sync.dma_start(wt[:], wv_d[ct * 128:(ct + 1) * 128, :])
                    wv_tiles.append(wt)
                for tt8 in range(8):
                    tt = th * 8 + tt8
                    pv = psQK.tile([128, 512], F32, tag="qkv")
                    for ct in range(NC_):
                        nc.tensor.matmul(
                            pv[:],
                            xT[:, ct, tt8 * 128:(tt8 + 1) * 128],
                            wv_tiles[ct][:],
                            start=(ct == 0), stop=(ct == NC_ - 1))
                    copy_any(
                        v_aug[:, :, tt, 0:64],
                        pv[:].rearrange("p (h d) -> p h d", h=H_PER_CORE))

        # preload the proj weights before attention so phase 3 starts hot
        wp_pool = ctx.enter_context(tc.tile_pool(name="wp", bufs=1))
        wp_sb = wp_pool.tile([128, 4, C], F32R)
        for kt in range(4):
            nc.sync.dma_start(wp_sb[:, kt, :], wp_d[kt * 128:(kt + 1) * 128, :])

        # ---- phase 2: attention ----
        with ExitStack() as p2:
            pT_pool = p2.enter_context(tc.tile_pool(name="pT", bufs=4))
            ySt_pool = p2.enter_context(tc.tile_pool(name="ySt", bufs=4))
            rc_pool = p2.enter_context(tc.tile_pool(name="rc", bufs=3))
            rb_pool = p2.enter_context(tc.tile_pool(name="rb", bufs=2))
            psS = p2.enter_context(tc.tile_pool(name="psS", bufs=3, space="PSUM"))
            psY = p2.enter_context(tc.tile_pool(name="psY", bufs=2, space="PSUM"))

            for h in range(H_PER_CORE):
                mt_h = h // 2
                for qsb in range(NQSB):
                    nkb = 4 * (qsb + 1)
                    yTp = psY.tile([65, 512], F32, tag="yTp")
                    # Full-width prefix blocks first (pairs sharing one
                    # 2-bank psum window and one batched exp), then the 4
                    # diagonal blocks, width-trimmed, paired two per window.
                    prefix = [(kb, 0) for kb in range(4 * qsb)]
                    diag = [(kb, kb * 128 - qsb * 512)
                            for kb in range(4 * qsb, nkb)]
                    groups = [prefix[i:i + 2] for i in range(0, len(prefix), 2)]
                    groups += [diag[0:2], diag[2:4]]
                    n_pv = 0
                    for g in groups:
                        wide = psS.tile([128, 1024], F32, tag="wide")
                        pTw = pT_pool.tile([128, 1024], F16, tag="pTw")
                        diag_group = g[0][0] >= 4 * qsb
                        for i, (kb, lo) in enumerate(g):
                            nc.tensor.matmul(
                                wide[:, i * 512 + lo:(i + 1) * 512],
                                kT_sb[:, mt_h, kb * 128:(kb + 1) * 128],
                                qT_pad[:, h, qsb * 512 + lo:(qsb + 1) * 512],
                                start=True, stop=True)
                        if diag_group:
                            # per-block exp, width-trimmed to the causal range
                            for i, (kb, lo) in enumerate(g):
                                nc.scalar.activation(
                                    pTw[:, i * 512 + lo:(i + 1) * 512],
                                    wide[:, i * 512 + lo:(i + 1) * 512],
                                    AF.Exp, bias=0.0, scale=SCALE)
                        else:
                            nc.scalar.activation(
                                pTw[:, 0:len(g) * 512], wide[:, 0:len(g) * 512],
                                AF.Exp, bias=0.0, scale=SCALE)
                        for i, (kb, lo) in enumerate(g):
                            if diag_group:
                                # zero the strictly-upper triangle
                                nc.vector.tensor_mul(
                                    pTw[:, i * 512 + lo:i * 512 + lo + 128],
                                    pTw[:, i * 512 + lo:i * 512 + lo + 128],
                                    tri_mask[:])
                            nc.tensor.matmul(
                                yTp[:, lo:512],
                                v_aug[:, h, kb, :],
                                pTw[:, i * 512 + lo:(i + 1) * 512],
                                start=(n_pv == 0), stop=(n_pv == nkb - 1))
                            n_pv += 1
                    # stage yT_aug out of PSUM immediately (frees the psum
                    # slot); the slow 1-lane reciprocal chain then runs off
                    # the critical path entirely in SBUF.
                    ySt = ySt_pool.tile([65, 512], F32, tag="ySt")
                    nc.vector.tensor_copy(ySt[:], yTp[:])
                    recip = rc_pool.tile([1, 512], F32)
                    nc.vector.reciprocal(recip[:], ySt[64:65, :])
                    rbc = rb_pool.tile([64, 512], F32)
                    nc.gpsimd.partition_broadcast(rbc[:], recip[:])
                    nc.vector.tensor_mul(
                        yT_sb[64 * (h % 2):64 * (h % 2) + 64, mt_h,
                              qsb * 512:(qsb + 1) * 512],
                        ySt[0:64, :], rbc[:])

        # ---- phase 3: output projection (partial; host sums head groups) ----
        with ExitStack() as p3:
            so_pool = p3.enter_context(tc.tile_pool(name="so", bufs=3))
            psO = p3.enter_context(tc.tile_pool(name="psO", bufs=4, space="PSUM"))
            for mt in range(NT):
                for n2 in range(2):
                    pp = psO.tile([128, 512], F32)
                    for kt in range(4):
                        nc.tensor.matmul(
                            pp[:],
                            yT_sb[:, kt, mt * 128:(mt + 1) * 128],
                            wp_sb[:, kt, n2 * 512:(n2 + 1) * 512],
                            start=(kt == 0), stop=(kt == 3))
                    so = so_pool.tile([128, 512], F32)
                    copy_any(so[:], pp[:])
                    nc.sync.dma_start(
                        out_d[mt * 128:(mt + 1) * 128, n2 * 512:(n2 + 1) * 512],
                        so[:])


_CACHE = {}


def _get_nc():
    if "nc" not in _CACHE:
        nc = bacc.Bacc("TRN2", target_bir_lowering=False, debug=False,
                       num_devices=N_CORES)
        build(nc)
        nc.compile()
        _CACHE["nc"] = nc
    return _CACHE["nc"]


def make_in_maps(x, w_attn, w_proj):
    x = np.asarray(x, dtype=np.float32)
    w_attn = np.asarray(w_attn, dtype=np.float32)
    w_proj = np.asarray(w_proj, dtype=np.float32)
    in_maps = []
    for core in range(N_CORES):
        b, hg = divmod(core, 2)
        cs = slice(hg * GC, (hg + 1) * GC)
        in_maps.append({
            "x": np.ascontiguousarray(x[b]),
            "wq": np.ascontiguousarray(w_attn[:, 0 * C:1 * C][:, cs]),
            "wk": np.ascontiguousarray(w_attn[:, 1 * C:2 * C][:, cs]),
            "wv": np.ascontiguousarray(w_attn[:, 2 * C:3 * C][:, cs]),
            "wp": np.ascontiguousarray(w_proj[cs, :]),
        })
    return in_maps


def kernel(x, w_attn, w_proj, _trace=False, _trace_kwargs=None):
    nc = _get_nc()
    in_maps = make_in_maps(x, w_attn, w_proj)
    res = run_bass_kernel_spmd(nc, in_maps, core_ids=list(range(N_CORES)),
                               trace=_trace, **(_trace_kwargs or {}))
    _CACHE["last_results"] = res
    B = np.asarray(x).shape[0]
    out = np.empty((B, T, C), dtype=np.float32)
    for b in range(B):
        out[b] = res.results[2 * b]["out"] + res.results[2 * b + 1]["out"]
    return out
